# revision 20
# baseline (speedup 1.0000x reference)
"""CLVP attention kernel for 8 Trainium2 NeuronCores (v2, software-pipelined).

Sharding: core c = 2*b + hg handles batch b (2048 tokens) and head-group hg
(8 of 16 heads).  Each core computes q/k/v projections for its heads, partial
rotary, attention, and a partial output projection over its heads' dims; the
host sums the two head-group partials per batch and adds the bias.

v2 highlights vs v1:
  - scores / attn@v / out-proj matmuls and their operands (qT/kT/ex/vext/
    o_n/m1) run in bf16 (same PE rate as f32r, ~0.3% extra error, well
    under the 2e-2 gate); projections stay f32r.
  - PE transposes write 4 pair-chunks into one scores-ring PSUM slot and a
    single strided DVE copy scatters them into bf16 qT/kT (no extra banks,
    no ACT copies).
  - ScalarE does ONLY the exp (the hard ~218us/core floor).
  - single tile scope, software-pipelined emission: k/v projections stream
    just-in-time inside the first attention loop, each (qc,p) group's
    scores+exp are emitted half a group (8 k-chunks) ahead of its attn@v,
    and q-proj / out-proj matmuls are spread into ACT-bound loops so the
    exp engine never starves.
"""

import numpy as np
import ml_dtypes

import concourse.bass as bass
import concourse.tile as tile
from concourse import bacc, mybir
from concourse.bass_utils import run_bass_kernel_spmd

B, S, E, H, D, ROT = 4, 2048, 1024, 16, 64, 32
HLOC = 8            # heads per core
HS = HLOC * D       # 512 head dims per core
N_CORES = 8
KE = E // 128       # 8 contraction tiles for projections
TT = S // 128       # 16 token tiles
QC = S // 512       # 4 q chunks
KC = S // 128       # 16 k chunks
PT = HS // 128      # 4 pair-tiles (2 heads each)

f32 = mybir.dt.float32
f32r = mybir.dt.float32r
bf16 = mybir.dt.bfloat16
FT = mybir.ActivationFunctionType

# scheduling features (bisectable)
LOOKAHEAD_ON = True
SPREAD_ON = True


def _emit(nc, tc, ctx, t, pfx=""):
    hidT, wq, wk, wv, m1, cmat, smat, ones_in, ident_in, part = t
    w_dram = {"q": wq, "k": wk, "v": wv}

    def tile_pool(name, **kw):
        return tc.tile_pool(name=pfx + name, **kw)

    const = ctx.enter_context(tile_pool(name="const", bufs=1))
    c_sb = const.tile([128, TT * 64], f32)
    nc.sync.dma_start(c_sb[:], cmat.ap())
    s_sb = const.tile([128, TT * 32], f32)
    nc.sync.dma_start(s_sb[:], smat.ap())
    # v_ext: [k-tile kc][head h][65] ; col 64 of each slot is 1.0 (softmax
    # denominator rides as row 64 of the attn@v accumulator)
    vext = const.tile([128, KC * HLOC * 65], bf16)
    nc.vector.memset(
        vext[:].rearrange("p (s c) -> p s c", c=65)[:, :, 64:65], 1.0
    )
    ones_t = const.tile([128, 64], f32r)
    nc.sync.dma_start(ones_t[:], ones_in.ap())
    ident = const.tile([128, 128], f32r)
    nc.sync.dma_start(ident[:], ident_in.ap())
    kT = const.tile([128, PT * S], bf16)   # [pair-tile][token]
    qT = const.tile([128, PT * S], bf16)
    m1_sb = const.tile([128, PT * E], bf16)

    w_pool = ctx.enter_context(tile_pool(name="wts", bufs=1))
    hid_pool = ctx.enter_context(tile_pool(name="hid", bufs=16))
    xnat_pool = ctx.enter_context(tile_pool(name="xnat", bufs=4))
    tmp_pool = ctx.enter_context(tile_pool(name="tmp", bufs=4))
    ex_pool = ctx.enter_context(tile_pool(name="exp", bufs=17))
    onorm_pool = ctx.enter_context(tile_pool(name="onorm", bufs=6))
    rz_pool = ctx.enter_context(tile_pool(name="rz", bufs=2))
    zsb_pool = ctx.enter_context(tile_pool(name="zsb", bufs=2))
    stg_pool = ctx.enter_context(tile_pool(name="stg", bufs=2))
    wos_pool = ctx.enter_context(tile_pool(name="wos", bufs=2))
    proj_psum = ctx.enter_context(tile_pool(name="pproj", bufs=2, space="PSUM"))
    s_psum = ctx.enter_context(tile_pool(name="ps_s", bufs=2, space="PSUM"))
    out_psum = ctx.enter_context(tile_pool(name="ps_o", bufs=2, space="PSUM"))

    pending_tr = []   # (X, tt, xn) tiles awaiting PE transpose into qT/kT

    # weight tiles; DMA'd in 128-row chunks so the first projection matmuls
    # can start before the full matrix lands
    w_sb = {}
    for name in ("q", "k", "v"):
        wt = w_pool.tile([128, KE * HS], f32r, name=f"w{name}", tag=f"w{name}")
        w_sb[name] = wt

    def dma_w(name):
        for k in range(KE):
            nc.sync.dma_start(
                w_sb[name][:, HS * k : HS * (k + 1)],
                w_dram[name].ap()[128 * k : 128 * (k + 1), :],
            )

    def dma_hid(c8):
        tiles = []
        for k in range(KE):
            ht = hid_pool.tile([128, 256], f32r, tag="hid")
            nc.sync.dma_start(
                ht[:], hidT.ap()[128 * k : 128 * (k + 1), 256 * c8 : 256 * c8 + 256]
            )
            tiles.append(ht)
        return tiles

    def proj_mms(X, tt, hid_sl):
        """The 8-matmul chain for one (projection, token-tile); returns psum."""
        t2 = tt % 2
        ps = proj_psum.tile([128, HS], f32, tag="pp")
        for k in range(KE):
            nc.tensor.matmul(
                ps[:],
                hid_sl[k][:, 128 * t2 : 128 * t2 + 128],
                w_sb[X][:, HS * k : HS * (k + 1)],
                start=(k == 0),
                stop=(k == KE - 1),
            )
        return ps

    def rotary_evict(X, tt, ps):
        """Partial-rotary + eviction of a projection psum tile; q/k also get
        DMA-XBAR transposed into qT/kT."""
        psv = ps[:].rearrange("p (h d) -> p h d", d=64)
        if X == "v":
            blk = vext[:, 520 * tt : 520 * (tt + 1)]
            outv = blk.rearrange("p (h c) -> p h c", c=65)[:, :, 0:64]
        else:
            xn = xnat_pool.tile([128, HS], f32r, tag="xn")
            outv = xn[:].rearrange("p (h d) -> p h d", d=64)
        cb = (
            c_sb[:, 64 * tt : 64 * (tt + 1)]
            .rearrange("p (o d) -> p o d", o=1)
            .broadcast_to([128, HLOC, 64])
        )
        nc.vector.tensor_mul(outv, psv, cb)
        tmp = tmp_pool.tile([128, 256], f32, tag="tmp")
        tmpv = tmp[:].rearrange("p (h d) -> p h d", d=32)
        s0 = (
            s_sb[:, 32 * tt : 32 * tt + 16]
            .rearrange("p (o d) -> p o d", o=1)
            .broadcast_to([128, HLOC, 16])
        )
        s1 = (
            s_sb[:, 32 * tt + 16 : 32 * tt + 32]
            .rearrange("p (o d) -> p o d", o=1)
            .broadcast_to([128, HLOC, 16])
        )
        nc.vector.tensor_mul(tmpv[:, :, 0:16], psv[:, :, 16:32], s0)
        nc.vector.tensor_mul(tmpv[:, :, 16:32], psv[:, :, 0:16], s1)
        rotslice = outv[:, :, 0:32]
        nc.vector.tensor_add(rotslice, rotslice, tmpv)
        if X != "v":
            pending_tr.append((X, tt, xn))

    def flush_tr():
        """PE-transpose the last projected q/k tile into qT/kT (lagged one
        tile so the rotary DVE ops are done by the time PE gets here)."""
        while pending_tr:
            X, tt, xn = pending_tr.pop(0)
            dest = kT if X == "k" else qT
            # scratch from the scores ring: keeps the proj ring's lag-2
            # double-buffering intact (a tp in "pp" would force lag-1 reuse)
            tp = s_psum.tile([128, 1024], f32r, tag="sps")
            for p in range(PT):
                nc.tensor.transpose(
                    tp[:, 128 * p : 128 * (p + 1)],
                    xn[:, 128 * p : 128 * (p + 1)],
                    ident[:],
                )
            nc.vector.tensor_copy(
                dest[:]
                .rearrange("r (p s) -> r p s", s=S)[:, :, 128 * tt : 128 * (tt + 1)],
                tp[:, 0:512].rearrange("r (p t) -> r p t", t=128),
            )

    def scores_exp(qc, p, kc):
        """One (qc, p, kc) scores pair + exp -> bf16 ex tile in SBUF."""
        sps = s_psum.tile([128, 1024], f32, tag="sps")
        nc.tensor.matmul(
            sps[:, 0:512],
            kT[0:64, S * p + 128 * kc : S * p + 128 * (kc + 1)],
            qT[0:64, S * p + 512 * qc : S * p + 512 * (qc + 1)],
            start=True,
            stop=True,
            tile_position=(0, 0),
        )
        nc.tensor.matmul(
            sps[:, 512:1024],
            kT[64:128, S * p + 128 * kc : S * p + 128 * (kc + 1)],
            qT[64:128, S * p + 512 * qc : S * p + 512 * (qc + 1)],
            start=True,
            stop=True,
            tile_position=(64, 0),
        )
        ex = ex_pool.tile([128, 1024], bf16, tag="ex")
        nc.scalar.activation(ex[:], sps[:], FT.Exp)
        return ex

    def attnv(p, kc, ex, outA, outB):
        nc.tensor.matmul(
            outA[:],
            vext[:, 520 * kc + 65 * (2 * p) : 520 * kc + 65 * (2 * p) + 65],
            ex[:, 0:512],
            start=(kc == 0),
            stop=(kc == KC - 1),
        )
        nc.tensor.matmul(
            outB[:],
            vext[:, 520 * kc + 65 * (2 * p + 1) : 520 * kc + 65 * (2 * p + 1) + 65],
            ex[:, 512:1024],
            start=(kc == 0),
            stop=(kc == KC - 1),
        )

    def normalize(outA, outB):
        """softmax denominators -> per-head normalized o_n [128, 512] bf16."""
        o_n = onorm_pool.tile([128, 512], bf16, tag="on")
        for hh, outps in ((0, outA), (1, outB)):
            rz = rz_pool.tile([128, 512], f32r, tag="rz")
            with nc.allow_low_precision(reason="f32r softmax denom recip"):
                nc.vector.reciprocal(rz[64:65, :], outps[64:65, :])
            zps = proj_psum.tile([64, 512], f32, tag="pp")
            nc.tensor.matmul(
                zps[:],
                ones_t[64:65, 0:64],
                rz[64:65, :],
                start=True,
                stop=True,
                tile_position=(64, 0),
            )
            zsb = zsb_pool.tile([64, 512], f32r, tag="zsb")
            nc.vector.tensor_copy(zsb[:], zps[:])
            if hh == 0:
                nc.vector.tensor_mul(o_n[0:64, :], outps[0:64, :], zsb[:])
            else:
                st = stg_pool.tile([64, 512], bf16, tag="st")
                nc.vector.tensor_mul(st[:], outps[0:64, :], zsb[:])
                nc.sync.dma_start(o_n[64:128, :], st[:])
        return o_n

    def wo_mm(qc, m, o_norm):
        wps = proj_psum.tile([128, 512], f32, tag="pp")
        for p in range(PT):
            nc.tensor.matmul(
                wps[:],
                m1_sb[:, E * p + 128 * m : E * p + 128 * (m + 1)],
                o_norm[p][:],
                start=(p == 0),
                stop=(p == PT - 1),
            )
        ws = wos_pool.tile([128, 512], f32, tag="ws")
        nc.vector.tensor_copy(ws[:], wps[:])
        nc.sync.dma_start(
            part.ap()[128 * m : 128 * (m + 1), 512 * qc : 512 * (qc + 1)], ws[:]
        )

    # ------------------------- emission schedule -------------------------
    dma_w("q")
    hid01 = [dma_hid(0), dma_hid(1)]
    dma_w("k")
    dma_w("v")

    # q-projection for qc=0 upfront (gates the first scores)
    for tt in range(4):
        ps = proj_mms("q", tt, hid01[tt // 2])
        flush_tr()
        rotary_evict("q", tt, ps)

    nc.sync.dma_start(m1_sb[:], m1.ap())

    # (pending_tr declared before first rotary_evict call)
    # groups processed in order; each group's scores+exp are emitted half a
    # group (8 k-chunks) ahead of its attn@v so the exp engine always has a
    # backlog and attn@v never waits on a fresh exp.
    LOOKAHEAD = KC // 2
    groups = [(qc, p) for qc in range(QC) for p in range(PT)]
    ex_tiles = {g: [] for g in groups}  # (qc,p) -> ex tiles in kc order
    o_norm = {}        # qc -> list of 4 o_n tiles
    hid_kv = [None]    # current kv hid slice group

    deferred_q = []    # list of (tt, hid_group) for spread q-proj
    deferred_wo = []   # list of (qc, m, o_norm) for spread out-proj

    def spread_one_q():
        if deferred_q:
            tt, hid_sl = deferred_q.pop(0)
            ps = proj_mms("q", tt, hid_sl)
            flush_tr()
            rotary_evict("q", tt, ps)
        elif pending_tr:
            flush_tr()

    def spread_one_wo():
        if deferred_wo:
            wqc, m, onr = deferred_wo.pop(0)
            wo_mm(wqc, m, onr)

    # fill loop: group 0 with just-in-time k/v projection; attn@v lags the
    # scores by one k-chunk so PE never waits on the exp of the same chunk
    g0 = groups[0]
    g1 = groups[1]
    for kc in range(KC + 1):
        if kc < KC:
            if kc % 2 == 0:
                hid_kv[0] = dma_hid(kc // 2)
            rotary_evict("k", kc, proj_mms("k", kc, hid_kv[0]))
            # v-proj matmuls cover the k-rotary DVE time, then the flush
            # transposes k(kc) into kT BEFORE the scores that read it
            rotary_evict("v", kc, proj_mms("v", kc, hid_kv[0]))
            flush_tr()
            ex_tiles[g0].append(scores_exp(*g0, kc))
            if LOOKAHEAD_ON and kc >= KC - LOOKAHEAD:
                ex_tiles[g1].append(scores_exp(*g1, kc - (KC - LOOKAHEAD)))
        if kc >= 1:
            if kc - 1 == 0:
                outA = out_psum.tile([65, 512], f32, tag="oA", bufs=1)
                outB = out_psum.tile([65, 512], f32, tag="oB", bufs=1)
            attnv(g0[1], kc - 1, ex_tiles[g0][kc - 1], outA, outB)
    flush_tr()
    o_norm.setdefault(g0[0], []).append(normalize(outA, outB))

    # steady-state loops: groups 1..15
    for gi in range(1, len(groups)):
        qc, p = groups[gi]
        cur = groups[gi]
        nxt = groups[gi + 1] if gi + 1 < len(groups) else None
        outA = out_psum.tile([65, 512], f32, tag="oA", bufs=1)
        outB = out_psum.tile([65, 512], f32, tag="oB", bufs=1)
        for kc in range(KC):
            # own late-half scores (first 8 iters), next group's early half
            # (last 8 iters) — keeps a steady ~8-tile exp backlog
            if LOOKAHEAD_ON:
                if kc < LOOKAHEAD:
                    ex_tiles[cur].append(scores_exp(qc, p, kc + LOOKAHEAD))
                elif nxt is not None:
                    ex_tiles[nxt].append(scores_exp(*nxt, kc - LOOKAHEAD))
            else:
                ex_tiles[cur].append(scores_exp(qc, p, kc))
            if SPREAD_ON:
                if p in (1, 2):
                    spread_one_q()
                if p in (0, 1):
                    spread_one_wo()
            attnv(p, kc, ex_tiles[cur][kc], outA, outB)
        flush_tr()
        ex_tiles[cur] = None
        o_norm.setdefault(qc, []).append(normalize(outA, outB))
        if p == 1 and qc < QC - 1:
            # stage next qc's q-projection inputs; MMs spread into p1/p2
            nq = qc + 1
            h0, h1 = dma_hid(2 * nq), dma_hid(2 * nq + 1)
            for tt in range(4 * nq, 4 * nq + 4):
                deferred_q.append((tt, h0 if tt % 4 < 2 else h1))
            if not SPREAD_ON:
                while deferred_q:
                    spread_one_q()
        if p == PT - 1:
            for m in range(E // 128):
                if qc == QC - 1 or not SPREAD_ON:
                    wo_mm(qc, m, o_norm[qc])
                else:
                    deferred_wo.append((qc, m, o_norm[qc]))
    while deferred_q:
        spread_one_q()
    while deferred_wo:
        spread_one_wo()


_NC_CACHE = {}


def _get_nc(reps=1):
    if reps in _NC_CACHE:
        return _NC_CACHE[reps]
    nc = bacc.Bacc("TRN2", target_bir_lowering=False, debug=False, num_devices=N_CORES)
    hidT = nc.dram_tensor("hidT", [E, S], f32r, kind="ExternalInput")
    wq = nc.dram_tensor("wq", [E, HS], f32r, kind="ExternalInput")
    wk = nc.dram_tensor("wk", [E, HS], f32r, kind="ExternalInput")
    wv = nc.dram_tensor("wv", [E, HS], f32r, kind="ExternalInput")
    m1 = nc.dram_tensor("m1", [128, PT * E], bf16, kind="ExternalInput")
    cmat = nc.dram_tensor("cmat", [128, TT * 64], f32, kind="ExternalInput")
    smat = nc.dram_tensor("smat", [128, TT * 32], f32, kind="ExternalInput")
    ones_in = nc.dram_tensor("ones", [128, 64], f32r, kind="ExternalInput")
    ident_in = nc.dram_tensor("ident", [128, 128], f32r, kind="ExternalInput")
    part = nc.dram_tensor("part", [E, S], f32, kind="ExternalOutput")
    from contextlib import ExitStack

    with tile.TileContext(nc) as tc:
        for r in range(reps):
            with ExitStack() as ctx:
                _emit(
                    nc, tc, ctx, (hidT, wq, wk, wv, m1, cmat, smat, ones_in, ident_in, part),
                    pfx=f"r{r}_" if reps > 1 else "",
                )
    nc.compile()
    _NC_CACHE[reps] = nc
    return nc


def _in_maps(hidden_states, rotary_pos_emb, Wq, Wk, Wv, Wo):
    scale = np.float32(D**-0.5)
    f = np.asarray(rotary_pos_emb, np.float32)[0]  # [S, ROT]
    cmat = np.ones((S, 64), np.float32)
    cmat[:, 0:ROT] = np.cos(f)
    smat = np.empty((S, ROT), np.float32)
    smat[:, 0:16] = -np.sin(f[:, 0:16])
    smat[:, 16:ROT] = np.sin(f[:, 16:ROT])
    hs = np.asarray(hidden_states, np.float32)
    Wq, Wk, Wv, Wo = (np.asarray(w, np.float32) for w in (Wq, Wk, Wv, Wo))
    maps = []
    for c in range(N_CORES):
        b, hg = divmod(c, 2)
        rows = slice(hg * HS, (hg + 1) * HS)
        maps.append(
            {
                "hidT": np.ascontiguousarray(hs[b].T),
                "wq": np.ascontiguousarray((Wq[rows] * scale).T),
                "wk": np.ascontiguousarray(Wk[rows].T),
                "wv": np.ascontiguousarray(Wv[rows].T),
                "m1": np.ascontiguousarray(
                    Wo[:, rows].T.reshape(PT, 128, E).transpose(1, 0, 2)
                    .reshape(128, PT * E)
                ).astype(ml_dtypes.bfloat16),
                "cmat": np.ascontiguousarray(
                    cmat.reshape(TT, 128, 64).transpose(1, 0, 2).reshape(128, TT * 64)
                ),
                "smat": np.ascontiguousarray(
                    smat.reshape(TT, 128, 32).transpose(1, 0, 2).reshape(128, TT * 32)
                ),
                "ones": np.ones((128, 64), np.float32),
                "ident": np.eye(128, dtype=np.float32),
            }
        )
    return maps


def kernel(hidden_states, rotary_pos_emb, Wq, Wk, Wv, Wo, bo, _trace=False):
    nc = _get_nc()
    maps = _in_maps(hidden_states, rotary_pos_emb, Wq, Wk, Wv, Wo)
    res = run_bass_kernel_spmd(
        nc, maps, core_ids=list(range(N_CORES)), trace=_trace
    )
    out = np.empty((B, S, E), np.float32)
    bo = np.asarray(bo, np.float32)
    for b in range(B):
        p0 = np.asarray(res.results[2 * b]["part"])
        p1 = np.asarray(res.results[2 * b + 1]["part"])
        out[b] = (p0 + p1).T + bo
    if _trace:
        kernel._last_results = res
    return out


# revision 21
# speedup vs baseline: 1.4584x; 1.4584x over previous
"""CLVP attention kernel for 8 Trainium2 NeuronCores (v2, software-pipelined).

Sharding: core c = 2*b + hg handles batch b (2048 tokens) and head-group hg
(8 of 16 heads).  Each core computes q/k/v projections for its heads, partial
rotary, attention, and a partial output projection over its heads' dims; the
host sums the two head-group partials per batch and adds the bias.

v2 highlights vs v1:
  - scores / attn@v / out-proj matmuls and their operands (qT/kT/ex/vext/
    o_n/m1) run in bf16 (same PE rate as f32r, ~0.3% extra error, well
    under the 2e-2 gate); projections stay f32r.
  - PE transposes write 4 pair-chunks into one scores-ring PSUM slot and a
    single strided DVE copy scatters them into bf16 qT/kT (no extra banks,
    no ACT copies).
  - ScalarE does ONLY the exp (the hard ~218us/core floor).
  - single tile scope, software-pipelined emission: k/v projections stream
    just-in-time inside the first attention loop, each (qc,p) group's
    scores+exp are emitted half a group (8 k-chunks) ahead of its attn@v,
    and q-proj / out-proj matmuls are spread into ACT-bound loops so the
    exp engine never starves.
"""

import numpy as np
import ml_dtypes

import concourse.bass as bass
import concourse.tile as tile
from concourse import bacc, mybir
from concourse.bass_utils import run_bass_kernel_spmd

B, S, E, H, D, ROT = 4, 2048, 1024, 16, 64, 32
HLOC = 8            # heads per core
HS = HLOC * D       # 512 head dims per core
N_CORES = 8
KE = E // 128       # 8 contraction tiles for projections
TT = S // 128       # 16 token tiles
QC = S // 512       # 4 q chunks
KC = S // 128       # 16 k chunks
PT = HS // 128      # 4 pair-tiles (2 heads each)

f32 = mybir.dt.float32
f32r = mybir.dt.float32r
bf16 = mybir.dt.bfloat16
FT = mybir.ActivationFunctionType

# scheduling features (bisectable)
LOOKAHEAD_ON = True
SPREAD_ON = True


def _emit(nc, tc, ctx, t, pfx=""):
    hidT, wq, wk, wv, m1, cmat, smat, ones_in, ident_in, part = t
    w_dram = {"q": wq, "k": wk, "v": wv}

    def tile_pool(name, **kw):
        return tc.tile_pool(name=pfx + name, **kw)

    const = ctx.enter_context(tile_pool(name="const", bufs=1))
    c_sb = const.tile([128, TT * 64], f32)
    nc.sync.dma_start(c_sb[:], cmat.ap())
    s_sb = const.tile([128, TT * 32], f32)
    nc.sync.dma_start(s_sb[:], smat.ap())
    # v_ext: [k-tile kc][head h][65] ; col 64 of each slot is 1.0 (softmax
    # denominator rides as row 64 of the attn@v accumulator)
    vext = const.tile([128, KC * HLOC * 65], bf16)
    nc.vector.memset(
        vext[:].rearrange("p (s c) -> p s c", c=65)[:, :, 64:65], 1.0
    )
    ones_t = const.tile([128, 64], f32r)
    nc.sync.dma_start(ones_t[:], ones_in.ap())
    ident = const.tile([128, 128], f32r)
    nc.sync.dma_start(ident[:], ident_in.ap())
    kT = const.tile([128, PT * S], bf16)   # [pair-tile][token]
    qT = const.tile([128, PT * S], bf16)
    m1_sb = const.tile([128, PT * E], bf16)

    w_pool = ctx.enter_context(tile_pool(name="wts", bufs=1))
    hid_pool = ctx.enter_context(tile_pool(name="hid", bufs=2))
    xnat_pool = ctx.enter_context(tile_pool(name="xnat", bufs=4))
    tmp_pool = ctx.enter_context(tile_pool(name="tmp", bufs=4))
    ex_pool = ctx.enter_context(tile_pool(name="exp", bufs=17))
    onorm_pool = ctx.enter_context(tile_pool(name="onorm", bufs=6))
    rz_pool = ctx.enter_context(tile_pool(name="rz", bufs=2))
    zsb_pool = ctx.enter_context(tile_pool(name="zsb", bufs=2))
    stg_pool = ctx.enter_context(tile_pool(name="stg", bufs=2))
    wos_pool = ctx.enter_context(tile_pool(name="wos", bufs=2))
    proj_psum = ctx.enter_context(tile_pool(name="pproj", bufs=2, space="PSUM"))
    s_psum = ctx.enter_context(tile_pool(name="ps_s", bufs=2, space="PSUM"))
    out_psum = ctx.enter_context(tile_pool(name="ps_o", bufs=2, space="PSUM"))

    pending_tr = []   # (X, tt, xn) tiles awaiting PE transpose into qT/kT

    # weight tiles; DMA'd in 128-row chunks so the first projection matmuls
    # can start before the full matrix lands
    w_sb = {}
    for name in ("q", "k", "v"):
        wt = w_pool.tile([128, KE * HS], f32r, name=f"w{name}", tag=f"w{name}")
        w_sb[name] = wt

    def dma_w(name):
        for k in range(KE):
            nc.sync.dma_start(
                w_sb[name][:, HS * k : HS * (k + 1)],
                w_dram[name].ap()[128 * k : 128 * (k + 1), :],
            )

    def dma_hid(c8):
        # one DMA per c8 group: all 8 E-chunks land side by side in a single
        # [128, 8*256] tile (8x fewer SP-seq DMA dispatches, same bytes)
        ht = hid_pool.tile([128, KE * 256], f32r, tag="hid")
        nc.sync.dma_start(
            ht[:].rearrange("p (k s) -> p k s", s=256),
            hidT.ap()
            .rearrange("(k p) s -> p k s", p=128)[:, :, 256 * c8 : 256 * c8 + 256],
        )
        return ht

    def proj_mms(X, tt, hid_sl):
        """The 8-matmul chain for one (projection, token-tile); returns psum."""
        t2 = tt % 2
        ps = proj_psum.tile([128, HS], f32, tag="pp")
        for k in range(KE):
            nc.tensor.matmul(
                ps[:],
                hid_sl[:, 256 * k + 128 * t2 : 256 * k + 128 * t2 + 128],
                w_sb[X][:, HS * k : HS * (k + 1)],
                start=(k == 0),
                stop=(k == KE - 1),
            )
        return ps

    def rotary_evict(X, tt, ps):
        """Partial-rotary + eviction of a projection psum tile; q/k also get
        DMA-XBAR transposed into qT/kT."""
        psv = ps[:].rearrange("p (h d) -> p h d", d=64)
        if X == "v":
            blk = vext[:, 520 * tt : 520 * (tt + 1)]
            outv = blk.rearrange("p (h c) -> p h c", c=65)[:, :, 0:64]
        else:
            xn = xnat_pool.tile([128, HS], f32r, tag="xn")
            outv = xn[:].rearrange("p (h d) -> p h d", d=64)
        cb = (
            c_sb[:, 64 * tt : 64 * (tt + 1)]
            .rearrange("p (o d) -> p o d", o=1)
            .broadcast_to([128, HLOC, 64])
        )
        nc.vector.tensor_mul(outv, psv, cb)
        tmp = tmp_pool.tile([128, 256], f32, tag="tmp")
        tmpv = tmp[:].rearrange("p (h d) -> p h d", d=32)
        s0 = (
            s_sb[:, 32 * tt : 32 * tt + 16]
            .rearrange("p (o d) -> p o d", o=1)
            .broadcast_to([128, HLOC, 16])
        )
        s1 = (
            s_sb[:, 32 * tt + 16 : 32 * tt + 32]
            .rearrange("p (o d) -> p o d", o=1)
            .broadcast_to([128, HLOC, 16])
        )
        nc.vector.tensor_mul(tmpv[:, :, 0:16], psv[:, :, 16:32], s0)
        nc.vector.tensor_mul(tmpv[:, :, 16:32], psv[:, :, 0:16], s1)
        rotslice = outv[:, :, 0:32]
        nc.vector.tensor_add(rotslice, rotslice, tmpv)
        if X != "v":
            pending_tr.append((X, tt, xn))

    def flush_tr():
        """PE-transpose the last projected q/k tile into qT/kT (lagged one
        tile so the rotary DVE ops are done by the time PE gets here)."""
        while pending_tr:
            X, tt, xn = pending_tr.pop(0)
            dest = kT if X == "k" else qT
            # scratch from the scores ring: keeps the proj ring's lag-2
            # double-buffering intact (a tp in "pp" would force lag-1 reuse)
            tp = s_psum.tile([128, 1024], f32r, tag="sps")
            for p in range(PT):
                nc.tensor.transpose(
                    tp[:, 128 * p : 128 * (p + 1)],
                    xn[:, 128 * p : 128 * (p + 1)],
                    ident[:],
                )
            nc.vector.tensor_copy(
                dest[:]
                .rearrange("r (p s) -> r p s", s=S)[:, :, 128 * tt : 128 * (tt + 1)],
                tp[:, 0:512].rearrange("r (p t) -> r p t", t=128),
            )

    def scores_exp(qc, p, kc):
        """One (qc, p, kc) scores pair + exp -> bf16 ex tile in SBUF."""
        sps = s_psum.tile([128, 1024], f32, tag="sps")
        nc.tensor.matmul(
            sps[:, 0:512],
            kT[0:64, S * p + 128 * kc : S * p + 128 * (kc + 1)],
            qT[0:64, S * p + 512 * qc : S * p + 512 * (qc + 1)],
            start=True,
            stop=True,
            tile_position=(0, 0),
        )
        nc.tensor.matmul(
            sps[:, 512:1024],
            kT[64:128, S * p + 128 * kc : S * p + 128 * (kc + 1)],
            qT[64:128, S * p + 512 * qc : S * p + 512 * (qc + 1)],
            start=True,
            stop=True,
            tile_position=(64, 0),
        )
        ex = ex_pool.tile([128, 1024], bf16, tag="ex")
        nc.scalar.activation(ex[:], sps[:], FT.Exp)
        return ex

    def attnv(p, kc, ex, outA, outB):
        nc.tensor.matmul(
            outA[:],
            vext[:, 520 * kc + 65 * (2 * p) : 520 * kc + 65 * (2 * p) + 65],
            ex[:, 0:512],
            start=(kc == 0),
            stop=(kc == KC - 1),
        )
        nc.tensor.matmul(
            outB[:],
            vext[:, 520 * kc + 65 * (2 * p + 1) : 520 * kc + 65 * (2 * p + 1) + 65],
            ex[:, 512:1024],
            start=(kc == 0),
            stop=(kc == KC - 1),
        )

    def normalize(outA, outB):
        """softmax denominators -> per-head normalized o_n [128, 512] bf16."""
        o_n = onorm_pool.tile([128, 512], bf16, tag="on")
        for hh, outps in ((0, outA), (1, outB)):
            rz = rz_pool.tile([128, 512], f32r, tag="rz")
            with nc.allow_low_precision(reason="f32r softmax denom recip"):
                nc.vector.reciprocal(rz[64:65, :], outps[64:65, :])
            zps = proj_psum.tile([64, 512], f32, tag="pp")
            nc.tensor.matmul(
                zps[:],
                ones_t[64:65, 0:64],
                rz[64:65, :],
                start=True,
                stop=True,
                tile_position=(64, 0),
            )
            zsb = zsb_pool.tile([64, 512], f32r, tag="zsb")
            nc.vector.tensor_copy(zsb[:], zps[:])
            if hh == 0:
                nc.vector.tensor_mul(o_n[0:64, :], outps[0:64, :], zsb[:])
            else:
                st = stg_pool.tile([64, 512], bf16, tag="st")
                nc.vector.tensor_mul(st[:], outps[0:64, :], zsb[:])
                nc.sync.dma_start(o_n[64:128, :], st[:])
        return o_n

    def wo_mm(qc, m, o_norm):
        wps = proj_psum.tile([128, 512], f32, tag="pp")
        for p in range(PT):
            nc.tensor.matmul(
                wps[:],
                m1_sb[:, E * p + 128 * m : E * p + 128 * (m + 1)],
                o_norm[p][:],
                start=(p == 0),
                stop=(p == PT - 1),
            )
        ws = wos_pool.tile([128, 512], f32, tag="ws")
        nc.vector.tensor_copy(ws[:], wps[:])
        nc.sync.dma_start(
            part.ap()[128 * m : 128 * (m + 1), 512 * qc : 512 * (qc + 1)], ws[:]
        )

    # ------------------------- emission schedule -------------------------
    dma_w("q")
    hid01 = [dma_hid(0), dma_hid(1)]
    dma_w("k")
    dma_w("v")

    # q-projection for qc=0 upfront (gates the first scores)
    for tt in range(4):
        ps = proj_mms("q", tt, hid01[tt // 2])
        flush_tr()
        rotary_evict("q", tt, ps)

    nc.sync.dma_start(m1_sb[:], m1.ap())

    # (pending_tr declared before first rotary_evict call)
    # groups processed in order; each group's scores+exp are emitted half a
    # group (8 k-chunks) ahead of its attn@v so the exp engine always has a
    # backlog and attn@v never waits on a fresh exp.
    LOOKAHEAD = KC // 2
    groups = [(qc, p) for qc in range(QC) for p in range(PT)]
    ex_tiles = {g: [] for g in groups}  # (qc,p) -> ex tiles in kc order
    o_norm = {}        # qc -> list of 4 o_n tiles
    hid_kv = [None]    # current kv hid slice group

    deferred_q = []    # list of (tt, hid_group) for spread q-proj
    deferred_wo = []   # list of (qc, m, o_norm) for spread out-proj

    def spread_one_q():
        if deferred_q:
            tt, hid_sl = deferred_q.pop(0)
            ps = proj_mms("q", tt, hid_sl)
            flush_tr()
            rotary_evict("q", tt, ps)
        elif pending_tr:
            flush_tr()

    def spread_one_wo():
        if deferred_wo:
            wqc, m, onr = deferred_wo.pop(0)
            wo_mm(wqc, m, onr)

    # fill loop: group 0 with just-in-time k/v projection; attn@v lags the
    # scores by one k-chunk so PE never waits on the exp of the same chunk
    g0 = groups[0]
    g1 = groups[1]
    for kc in range(KC + 1):
        if kc < KC:
            if kc % 2 == 0:
                hid_kv[0] = dma_hid(kc // 2)
            rotary_evict("k", kc, proj_mms("k", kc, hid_kv[0]))
            # v-proj matmuls cover the k-rotary DVE time, then the flush
            # transposes k(kc) into kT BEFORE the scores that read it
            rotary_evict("v", kc, proj_mms("v", kc, hid_kv[0]))
            flush_tr()
            ex_tiles[g0].append(scores_exp(*g0, kc))
            if LOOKAHEAD_ON and kc >= KC - LOOKAHEAD:
                ex_tiles[g1].append(scores_exp(*g1, kc - (KC - LOOKAHEAD)))
        if kc >= 1:
            if kc - 1 == 0:
                outA = out_psum.tile([65, 512], f32, tag="oA", bufs=1)
                outB = out_psum.tile([65, 512], f32, tag="oB", bufs=1)
            attnv(g0[1], kc - 1, ex_tiles[g0][kc - 1], outA, outB)
    flush_tr()
    o_norm.setdefault(g0[0], []).append(normalize(outA, outB))

    # steady-state loops: groups 1..15
    for gi in range(1, len(groups)):
        qc, p = groups[gi]
        cur = groups[gi]
        nxt = groups[gi + 1] if gi + 1 < len(groups) else None
        outA = out_psum.tile([65, 512], f32, tag="oA", bufs=1)
        outB = out_psum.tile([65, 512], f32, tag="oB", bufs=1)
        for kc in range(KC):
            # own late-half scores (first 8 iters), next group's early half
            # (last 8 iters) — keeps a steady ~8-tile exp backlog
            if LOOKAHEAD_ON:
                if kc < LOOKAHEAD:
                    ex_tiles[cur].append(scores_exp(qc, p, kc + LOOKAHEAD))
                elif nxt is not None:
                    ex_tiles[nxt].append(scores_exp(*nxt, kc - LOOKAHEAD))
            else:
                ex_tiles[cur].append(scores_exp(qc, p, kc))
            if SPREAD_ON:
                if p in (1, 2):
                    spread_one_q()
                if p in (0, 1):
                    spread_one_wo()
            attnv(p, kc, ex_tiles[cur][kc], outA, outB)
        flush_tr()
        ex_tiles[cur] = None
        o_norm.setdefault(qc, []).append(normalize(outA, outB))
        if p == 1 and qc < QC - 1:
            # stage next qc's q-projection inputs; MMs spread into p1/p2
            nq = qc + 1
            h0, h1 = dma_hid(2 * nq), dma_hid(2 * nq + 1)
            for tt in range(4 * nq, 4 * nq + 4):
                deferred_q.append((tt, h0 if tt % 4 < 2 else h1))
            if not SPREAD_ON:
                while deferred_q:
                    spread_one_q()
        if p == PT - 1:
            for m in range(E // 128):
                if qc == QC - 1 or not SPREAD_ON:
                    wo_mm(qc, m, o_norm[qc])
                else:
                    deferred_wo.append((qc, m, o_norm[qc]))
    while deferred_q:
        spread_one_q()
    while deferred_wo:
        spread_one_wo()


_NC_CACHE = {}


def _get_nc(reps=1):
    if reps in _NC_CACHE:
        return _NC_CACHE[reps]
    nc = bacc.Bacc("TRN2", target_bir_lowering=False, debug=False, num_devices=N_CORES)
    hidT = nc.dram_tensor("hidT", [E, S], f32r, kind="ExternalInput")
    wq = nc.dram_tensor("wq", [E, HS], f32r, kind="ExternalInput")
    wk = nc.dram_tensor("wk", [E, HS], f32r, kind="ExternalInput")
    wv = nc.dram_tensor("wv", [E, HS], f32r, kind="ExternalInput")
    m1 = nc.dram_tensor("m1", [128, PT * E], bf16, kind="ExternalInput")
    cmat = nc.dram_tensor("cmat", [128, TT * 64], f32, kind="ExternalInput")
    smat = nc.dram_tensor("smat", [128, TT * 32], f32, kind="ExternalInput")
    ones_in = nc.dram_tensor("ones", [128, 64], f32r, kind="ExternalInput")
    ident_in = nc.dram_tensor("ident", [128, 128], f32r, kind="ExternalInput")
    part = nc.dram_tensor("part", [E, S], f32, kind="ExternalOutput")
    from contextlib import ExitStack

    with tile.TileContext(nc) as tc:
        for r in range(reps):
            with ExitStack() as ctx:
                _emit(
                    nc, tc, ctx, (hidT, wq, wk, wv, m1, cmat, smat, ones_in, ident_in, part),
                    pfx=f"r{r}_" if reps > 1 else "",
                )
    nc.compile()
    _NC_CACHE[reps] = nc
    return nc


def _in_maps(hidden_states, rotary_pos_emb, Wq, Wk, Wv, Wo):
    scale = np.float32(D**-0.5)
    f = np.asarray(rotary_pos_emb, np.float32)[0]  # [S, ROT]
    cmat = np.ones((S, 64), np.float32)
    cmat[:, 0:ROT] = np.cos(f)
    smat = np.empty((S, ROT), np.float32)
    smat[:, 0:16] = -np.sin(f[:, 0:16])
    smat[:, 16:ROT] = np.sin(f[:, 16:ROT])
    hs = np.asarray(hidden_states, np.float32)
    Wq, Wk, Wv, Wo = (np.asarray(w, np.float32) for w in (Wq, Wk, Wv, Wo))
    maps = []
    for c in range(N_CORES):
        b, hg = divmod(c, 2)
        rows = slice(hg * HS, (hg + 1) * HS)
        maps.append(
            {
                "hidT": np.ascontiguousarray(hs[b].T),
                "wq": np.ascontiguousarray((Wq[rows] * scale).T),
                "wk": np.ascontiguousarray(Wk[rows].T),
                "wv": np.ascontiguousarray(Wv[rows].T),
                "m1": np.ascontiguousarray(
                    Wo[:, rows].T.reshape(PT, 128, E).transpose(1, 0, 2)
                    .reshape(128, PT * E)
                ).astype(ml_dtypes.bfloat16),
                "cmat": np.ascontiguousarray(
                    cmat.reshape(TT, 128, 64).transpose(1, 0, 2).reshape(128, TT * 64)
                ),
                "smat": np.ascontiguousarray(
                    smat.reshape(TT, 128, 32).transpose(1, 0, 2).reshape(128, TT * 32)
                ),
                "ones": np.ones((128, 64), np.float32),
                "ident": np.eye(128, dtype=np.float32),
            }
        )
    return maps


def kernel(hidden_states, rotary_pos_emb, Wq, Wk, Wv, Wo, bo, _trace=False):
    nc = _get_nc()
    maps = _in_maps(hidden_states, rotary_pos_emb, Wq, Wk, Wv, Wo)
    res = run_bass_kernel_spmd(
        nc, maps, core_ids=list(range(N_CORES)), trace=_trace
    )
    out = np.empty((B, S, E), np.float32)
    bo = np.asarray(bo, np.float32)
    for b in range(B):
        p0 = np.asarray(res.results[2 * b]["part"])
        p1 = np.asarray(res.results[2 * b + 1]["part"])
        out[b] = (p0 + p1).T + bo
    if _trace:
        kernel._last_results = res
    return out


# revision 22
# speedup vs baseline: 1.7033x; 1.1679x over previous
"""CLVP attention kernel for 8 Trainium2 NeuronCores (v2, software-pipelined).

Sharding: core c = 2*b + hg handles batch b (2048 tokens) and head-group hg
(8 of 16 heads).  Each core computes q/k/v projections for its heads, partial
rotary, attention, and a partial output projection over its heads' dims; the
host sums the two head-group partials per batch and adds the bias.

v2 highlights vs v1:
  - scores / attn@v / out-proj matmuls and their operands (qT/kT/ex/vext/
    o_n/m1) run in bf16 (same PE rate as f32r, ~0.3% extra error, well
    under the 2e-2 gate); projections stay f32r.
  - PE transposes write 4 pair-chunks into one scores-ring PSUM slot and a
    single strided DVE copy scatters them into bf16 qT/kT (no extra banks,
    no ACT copies).
  - ScalarE does ONLY the exp (the hard ~218us/core floor).
  - single tile scope, software-pipelined emission: k/v projections stream
    just-in-time inside the first attention loop, each (qc,p) group's
    scores+exp are emitted half a group (8 k-chunks) ahead of its attn@v,
    and q-proj / out-proj matmuls are spread into ACT-bound loops so the
    exp engine never starves.
"""

import numpy as np
import ml_dtypes

import concourse.bass as bass
import concourse.tile as tile
from concourse import bacc, mybir
from concourse.bass_utils import run_bass_kernel_spmd

B, S, E, H, D, ROT = 4, 2048, 1024, 16, 64, 32
HLOC = 8            # heads per core
HS = HLOC * D       # 512 head dims per core
N_CORES = 8
KE = E // 128       # 8 contraction tiles for projections
TT = S // 128       # 16 token tiles
QC = S // 512       # 4 q chunks
KC = S // 128       # 16 k chunks
PT = HS // 128      # 4 pair-tiles (2 heads each)

f32 = mybir.dt.float32
f32r = mybir.dt.float32r
bf16 = mybir.dt.bfloat16
FT = mybir.ActivationFunctionType

# scheduling features (bisectable)
LOOKAHEAD_ON = True
SPREAD_ON = True


def _emit(nc, tc, ctx, t, pfx=""):
    hidT, wq, wk, wv, m1, cmat, smat, ones_in, ident_in, part = t
    w_dram = {"q": wq, "k": wk, "v": wv}

    def tile_pool(name, **kw):
        return tc.tile_pool(name=pfx + name, **kw)

    const = ctx.enter_context(tile_pool(name="const", bufs=1))
    c_sb = const.tile([128, TT * 64], f32)
    nc.sync.dma_start(c_sb[:], cmat.ap())
    s_sb = const.tile([128, TT * 32], f32)
    nc.sync.dma_start(s_sb[:], smat.ap())
    # v_ext: [k-tile kc][head h][65] ; col 64 of each slot is 1.0 (softmax
    # denominator rides as row 64 of the attn@v accumulator)
    vext = const.tile([128, KC * HLOC * 65], bf16)
    nc.vector.memset(
        vext[:].rearrange("p (s c) -> p s c", c=65)[:, :, 64:65], 1.0
    )
    ones_t = const.tile([128, 64], f32r)
    nc.sync.dma_start(ones_t[:], ones_in.ap())
    ident = const.tile([128, 128], f32r)
    nc.sync.dma_start(ident[:], ident_in.ap())
    kT = const.tile([128, PT * S], bf16)   # [pair-tile][token]
    qT = const.tile([128, PT * S], bf16)
    m1_sb = const.tile([128, PT * E], bf16)

    w_pool = ctx.enter_context(tile_pool(name="wts", bufs=1))
    hid_pool = ctx.enter_context(tile_pool(name="hid", bufs=2))
    xnat_pool = ctx.enter_context(tile_pool(name="xnat", bufs=4))
    tmp_pool = ctx.enter_context(tile_pool(name="tmp", bufs=4))
    ex_pool = ctx.enter_context(tile_pool(name="exp", bufs=17))
    onorm_pool = ctx.enter_context(tile_pool(name="onorm", bufs=6))
    rz_pool = ctx.enter_context(tile_pool(name="rz", bufs=2))
    zsb_pool = ctx.enter_context(tile_pool(name="zsb", bufs=2))
    stg_pool = ctx.enter_context(tile_pool(name="stg", bufs=2))
    wos_pool = ctx.enter_context(tile_pool(name="wos", bufs=2))
    proj_psum = ctx.enter_context(tile_pool(name="pproj", bufs=2, space="PSUM"))
    s_psum = ctx.enter_context(tile_pool(name="ps_s", bufs=2, space="PSUM"))
    out_psum = ctx.enter_context(tile_pool(name="ps_o", bufs=2, space="PSUM"))

    pending_tr = []   # (X, tt, xn) tiles awaiting PE transpose into qT/kT

    # weight tiles; DMA'd in 128-row chunks so the first projection matmuls
    # can start before the full matrix lands
    w_sb = {}
    for name in ("q", "k", "v"):
        wt = w_pool.tile([128, KE * HS], f32r, name=f"w{name}", tag=f"w{name}")
        w_sb[name] = wt

    def dma_w(name):
        nc.sync.dma_start(
            w_sb[name][:].rearrange("p (k n) -> p k n", n=HS),
            w_dram[name].ap().rearrange("(k p) n -> p k n", p=128),
        )

    def dma_hid(c8):
        # one DMA per c8 group: all 8 E-chunks land side by side in a single
        # [128, 8*256] tile (8x fewer SP-seq DMA dispatches, same bytes)
        ht = hid_pool.tile([128, KE * 256], f32r, tag="hid")
        nc.sync.dma_start(
            ht[:].rearrange("p (k s) -> p k s", s=256),
            hidT.ap()
            .rearrange("(k p) s -> p k s", p=128)[:, :, 256 * c8 : 256 * c8 + 256],
        )
        return ht

    def proj_mms(X, tt, hid_sl):
        """The 8-matmul chain for one (projection, token-tile); returns psum."""
        t2 = tt % 2
        ps = proj_psum.tile([128, HS], f32, tag="pp")
        for k in range(KE):
            nc.tensor.matmul(
                ps[:],
                hid_sl[:, 256 * k + 128 * t2 : 256 * k + 128 * t2 + 128],
                w_sb[X][:, HS * k : HS * (k + 1)],
                start=(k == 0),
                stop=(k == KE - 1),
            )
        return ps

    def rotary_evict(X, tt, ps):
        """Partial-rotary + eviction of a projection psum tile; q/k also get
        DMA-XBAR transposed into qT/kT."""
        psv = ps[:].rearrange("p (h d) -> p h d", d=64)
        if X == "v":
            blk = vext[:, 520 * tt : 520 * (tt + 1)]
            outv = blk.rearrange("p (h c) -> p h c", c=65)[:, :, 0:64]
        else:
            xn = xnat_pool.tile([128, HS], f32r, tag="xn")
            outv = xn[:].rearrange("p (h d) -> p h d", d=64)
        cb = (
            c_sb[:, 64 * tt : 64 * (tt + 1)]
            .rearrange("p (o d) -> p o d", o=1)
            .broadcast_to([128, HLOC, 64])
        )
        nc.vector.tensor_mul(outv, psv, cb)
        tmp = tmp_pool.tile([128, 256], f32, tag="tmp")
        tmpv = tmp[:].rearrange("p (h d) -> p h d", d=32)
        s0 = (
            s_sb[:, 32 * tt : 32 * tt + 16]
            .rearrange("p (o d) -> p o d", o=1)
            .broadcast_to([128, HLOC, 16])
        )
        s1 = (
            s_sb[:, 32 * tt + 16 : 32 * tt + 32]
            .rearrange("p (o d) -> p o d", o=1)
            .broadcast_to([128, HLOC, 16])
        )
        nc.vector.tensor_mul(tmpv[:, :, 0:16], psv[:, :, 16:32], s0)
        nc.vector.tensor_mul(tmpv[:, :, 16:32], psv[:, :, 0:16], s1)
        rotslice = outv[:, :, 0:32]
        nc.vector.tensor_add(rotslice, rotslice, tmpv)
        if X != "v":
            pending_tr.append((X, tt, xn))

    def flush_tr():
        """PE-transpose the last projected q/k tile into qT/kT (lagged one
        tile so the rotary DVE ops are done by the time PE gets here)."""
        while pending_tr:
            X, tt, xn = pending_tr.pop(0)
            dest = kT if X == "k" else qT
            # scratch from the scores ring: keeps the proj ring's lag-2
            # double-buffering intact (a tp in "pp" would force lag-1 reuse)
            tp = s_psum.tile([128, 1024], f32r, tag="sps")
            for p in range(PT):
                nc.tensor.transpose(
                    tp[:, 128 * p : 128 * (p + 1)],
                    xn[:, 128 * p : 128 * (p + 1)],
                    ident[:],
                )
            nc.vector.tensor_copy(
                dest[:]
                .rearrange("r (p s) -> r p s", s=S)[:, :, 128 * tt : 128 * (tt + 1)],
                tp[:, 0:512].rearrange("r (p t) -> r p t", t=128),
            )

    def scores_exp(qc, p, kc):
        """One (qc, p, kc) scores pair + exp -> bf16 ex tile in SBUF."""
        sps = s_psum.tile([128, 1024], f32, tag="sps")
        nc.tensor.matmul(
            sps[:, 0:512],
            kT[0:64, S * p + 128 * kc : S * p + 128 * (kc + 1)],
            qT[0:64, S * p + 512 * qc : S * p + 512 * (qc + 1)],
            start=True,
            stop=True,
            tile_position=(0, 0),
        )
        nc.tensor.matmul(
            sps[:, 512:1024],
            kT[64:128, S * p + 128 * kc : S * p + 128 * (kc + 1)],
            qT[64:128, S * p + 512 * qc : S * p + 512 * (qc + 1)],
            start=True,
            stop=True,
            tile_position=(64, 0),
        )
        ex = ex_pool.tile([128, 1024], bf16, tag="ex")
        nc.scalar.activation(ex[:], sps[:], FT.Exp)
        return ex

    def attnv(p, kc, ex, outA, outB):
        nc.tensor.matmul(
            outA[:],
            vext[:, 520 * kc + 65 * (2 * p) : 520 * kc + 65 * (2 * p) + 65],
            ex[:, 0:512],
            start=(kc == 0),
            stop=(kc == KC - 1),
        )
        nc.tensor.matmul(
            outB[:],
            vext[:, 520 * kc + 65 * (2 * p + 1) : 520 * kc + 65 * (2 * p + 1) + 65],
            ex[:, 512:1024],
            start=(kc == 0),
            stop=(kc == KC - 1),
        )

    def normalize(outA, outB):
        """softmax denominators -> per-head normalized o_n [128, 512] bf16."""
        o_n = onorm_pool.tile([128, 512], bf16, tag="on")
        for hh, outps in ((0, outA), (1, outB)):
            rz = rz_pool.tile([128, 512], f32r, tag="rz")
            with nc.allow_low_precision(reason="f32r softmax denom recip"):
                nc.vector.reciprocal(rz[64:65, :], outps[64:65, :])
            zps = proj_psum.tile([64, 512], f32, tag="pp")
            nc.tensor.matmul(
                zps[:],
                ones_t[64:65, 0:64],
                rz[64:65, :],
                start=True,
                stop=True,
                tile_position=(64, 0),
            )
            zsb = zsb_pool.tile([64, 512], f32r, tag="zsb")
            nc.vector.tensor_copy(zsb[:], zps[:])
            if hh == 0:
                nc.vector.tensor_mul(o_n[0:64, :], outps[0:64, :], zsb[:])
            else:
                st = stg_pool.tile([64, 512], bf16, tag="st")
                nc.vector.tensor_mul(st[:], outps[0:64, :], zsb[:])
                nc.sync.dma_start(o_n[64:128, :], st[:])
        return o_n

    def wo_mm(qc, m, o_norm):
        wps = proj_psum.tile([128, 512], f32, tag="pp")
        for p in range(PT):
            nc.tensor.matmul(
                wps[:],
                m1_sb[:, E * p + 128 * m : E * p + 128 * (m + 1)],
                o_norm[p][:],
                start=(p == 0),
                stop=(p == PT - 1),
            )
        ws = wos_pool.tile([128, 512], f32, tag="ws")
        nc.vector.tensor_copy(ws[:], wps[:])
        nc.sync.dma_start(
            part.ap()[128 * m : 128 * (m + 1), 512 * qc : 512 * (qc + 1)], ws[:]
        )

    # ------------------------- emission schedule -------------------------
    dma_w("q")
    hid01 = [dma_hid(0), dma_hid(1)]
    dma_w("k")
    dma_w("v")

    # q-projection for qc=0 upfront (gates the first scores)
    for tt in range(4):
        ps = proj_mms("q", tt, hid01[tt // 2])
        flush_tr()
        rotary_evict("q", tt, ps)

    nc.sync.dma_start(m1_sb[:], m1.ap())

    # (pending_tr declared before first rotary_evict call)
    # groups processed in order; each group's scores+exp are emitted half a
    # group (8 k-chunks) ahead of its attn@v so the exp engine always has a
    # backlog and attn@v never waits on a fresh exp.
    LOOKAHEAD = KC // 2
    groups = [(qc, p) for qc in range(QC) for p in range(PT)]
    ex_tiles = {g: [] for g in groups}  # (qc,p) -> ex tiles in kc order
    o_norm = {}        # qc -> list of 4 o_n tiles
    hid_kv = [None]    # current kv hid slice group

    deferred_q = []    # list of (tt, hid_group) for spread q-proj
    deferred_wo = []   # list of (qc, m, o_norm) for spread out-proj

    def spread_one_q():
        if deferred_q:
            tt, hid_sl = deferred_q.pop(0)
            ps = proj_mms("q", tt, hid_sl)
            flush_tr()
            rotary_evict("q", tt, ps)
        elif pending_tr:
            flush_tr()

    def spread_one_wo():
        if deferred_wo:
            wqc, m, onr = deferred_wo.pop(0)
            wo_mm(wqc, m, onr)

    # fill loop: group 0 with just-in-time k/v projection; attn@v lags the
    # scores by one k-chunk so PE never waits on the exp of the same chunk
    g0 = groups[0]
    g1 = groups[1]
    for kc in range(KC + 1):
        if kc < KC:
            if kc % 2 == 0:
                hid_kv[0] = dma_hid(kc // 2)
            rotary_evict("k", kc, proj_mms("k", kc, hid_kv[0]))
            # v-proj matmuls cover the k-rotary DVE time, then the flush
            # transposes k(kc) into kT BEFORE the scores that read it
            rotary_evict("v", kc, proj_mms("v", kc, hid_kv[0]))
            flush_tr()
            ex_tiles[g0].append(scores_exp(*g0, kc))
            if LOOKAHEAD_ON and kc >= KC - LOOKAHEAD:
                ex_tiles[g1].append(scores_exp(*g1, kc - (KC - LOOKAHEAD)))
        if kc >= 1:
            if kc - 1 == 0:
                outA = out_psum.tile([65, 512], f32, tag="oA", bufs=1)
                outB = out_psum.tile([65, 512], f32, tag="oB", bufs=1)
            attnv(g0[1], kc - 1, ex_tiles[g0][kc - 1], outA, outB)
    flush_tr()
    o_norm.setdefault(g0[0], []).append(normalize(outA, outB))

    # steady-state loops: groups 1..15
    for gi in range(1, len(groups)):
        qc, p = groups[gi]
        cur = groups[gi]
        nxt = groups[gi + 1] if gi + 1 < len(groups) else None
        outA = out_psum.tile([65, 512], f32, tag="oA", bufs=1)
        outB = out_psum.tile([65, 512], f32, tag="oB", bufs=1)
        for kc in range(KC):
            # own late-half scores (first 8 iters), next group's early half
            # (last 8 iters) — keeps a steady ~8-tile exp backlog
            if LOOKAHEAD_ON:
                if kc < LOOKAHEAD:
                    ex_tiles[cur].append(scores_exp(qc, p, kc + LOOKAHEAD))
                elif nxt is not None:
                    ex_tiles[nxt].append(scores_exp(*nxt, kc - LOOKAHEAD))
            else:
                ex_tiles[cur].append(scores_exp(qc, p, kc))
            if SPREAD_ON:
                if p in (1, 2):
                    spread_one_q()
                if p in (0, 1):
                    spread_one_wo()
            attnv(p, kc, ex_tiles[cur][kc], outA, outB)
        flush_tr()
        ex_tiles[cur] = None
        o_norm.setdefault(qc, []).append(normalize(outA, outB))
        if p == 1 and qc < QC - 1:
            # stage next qc's q-projection inputs; MMs spread into p1/p2
            nq = qc + 1
            h0, h1 = dma_hid(2 * nq), dma_hid(2 * nq + 1)
            for tt in range(4 * nq, 4 * nq + 4):
                deferred_q.append((tt, h0 if tt % 4 < 2 else h1))
            if not SPREAD_ON:
                while deferred_q:
                    spread_one_q()
        if p == PT - 1:
            for m in range(E // 128):
                if qc == QC - 1 or not SPREAD_ON:
                    wo_mm(qc, m, o_norm[qc])
                else:
                    deferred_wo.append((qc, m, o_norm[qc]))
    while deferred_q:
        spread_one_q()
    while deferred_wo:
        spread_one_wo()


_NC_CACHE = {}


def _get_nc(reps=1):
    if reps in _NC_CACHE:
        return _NC_CACHE[reps]
    nc = bacc.Bacc("TRN2", target_bir_lowering=False, debug=False, num_devices=N_CORES)
    hidT = nc.dram_tensor("hidT", [E, S], f32r, kind="ExternalInput")
    wq = nc.dram_tensor("wq", [E, HS], f32r, kind="ExternalInput")
    wk = nc.dram_tensor("wk", [E, HS], f32r, kind="ExternalInput")
    wv = nc.dram_tensor("wv", [E, HS], f32r, kind="ExternalInput")
    m1 = nc.dram_tensor("m1", [128, PT * E], bf16, kind="ExternalInput")
    cmat = nc.dram_tensor("cmat", [128, TT * 64], f32, kind="ExternalInput")
    smat = nc.dram_tensor("smat", [128, TT * 32], f32, kind="ExternalInput")
    ones_in = nc.dram_tensor("ones", [128, 64], f32r, kind="ExternalInput")
    ident_in = nc.dram_tensor("ident", [128, 128], f32r, kind="ExternalInput")
    part = nc.dram_tensor("part", [E, S], f32, kind="ExternalOutput")
    from contextlib import ExitStack

    with tile.TileContext(nc) as tc:
        for r in range(reps):
            with ExitStack() as ctx:
                _emit(
                    nc, tc, ctx, (hidT, wq, wk, wv, m1, cmat, smat, ones_in, ident_in, part),
                    pfx=f"r{r}_" if reps > 1 else "",
                )
    nc.compile()
    _NC_CACHE[reps] = nc
    return nc


def _in_maps(hidden_states, rotary_pos_emb, Wq, Wk, Wv, Wo):
    scale = np.float32(D**-0.5)
    f = np.asarray(rotary_pos_emb, np.float32)[0]  # [S, ROT]
    cmat = np.ones((S, 64), np.float32)
    cmat[:, 0:ROT] = np.cos(f)
    smat = np.empty((S, ROT), np.float32)
    smat[:, 0:16] = -np.sin(f[:, 0:16])
    smat[:, 16:ROT] = np.sin(f[:, 16:ROT])
    hs = np.asarray(hidden_states, np.float32)
    Wq, Wk, Wv, Wo = (np.asarray(w, np.float32) for w in (Wq, Wk, Wv, Wo))
    maps = []
    for c in range(N_CORES):
        b, hg = divmod(c, 2)
        rows = slice(hg * HS, (hg + 1) * HS)
        maps.append(
            {
                "hidT": np.ascontiguousarray(hs[b].T),
                "wq": np.ascontiguousarray((Wq[rows] * scale).T),
                "wk": np.ascontiguousarray(Wk[rows].T),
                "wv": np.ascontiguousarray(Wv[rows].T),
                "m1": np.ascontiguousarray(
                    Wo[:, rows].T.reshape(PT, 128, E).transpose(1, 0, 2)
                    .reshape(128, PT * E)
                ).astype(ml_dtypes.bfloat16),
                "cmat": np.ascontiguousarray(
                    cmat.reshape(TT, 128, 64).transpose(1, 0, 2).reshape(128, TT * 64)
                ),
                "smat": np.ascontiguousarray(
                    smat.reshape(TT, 128, 32).transpose(1, 0, 2).reshape(128, TT * 32)
                ),
                "ones": np.ones((128, 64), np.float32),
                "ident": np.eye(128, dtype=np.float32),
            }
        )
    return maps


def kernel(hidden_states, rotary_pos_emb, Wq, Wk, Wv, Wo, bo, _trace=False):
    nc = _get_nc()
    maps = _in_maps(hidden_states, rotary_pos_emb, Wq, Wk, Wv, Wo)
    res = run_bass_kernel_spmd(
        nc, maps, core_ids=list(range(N_CORES)), trace=_trace
    )
    out = np.empty((B, S, E), np.float32)
    bo = np.asarray(bo, np.float32)
    for b in range(B):
        p0 = np.asarray(res.results[2 * b]["part"])
        p1 = np.asarray(res.results[2 * b + 1]["part"])
        out[b] = (p0 + p1).T + bo
    if _trace:
        kernel._last_results = res
    return out
